# revision 13
# baseline (speedup 1.0000x reference)
"""DiffusionConv (K=3) Bass kernel for 8 Trainium2 NeuronCores.

Strategy (node-range sharding, v3):
  - Nodes are split into two classes of exactly 25088; class-0 lives on cores
    0-3 (new ids < 25088), class-1 on cores 4-7. Within each class, a greedy
    balancer packs 196 blocks of 128 nodes with per-source-class in-edge caps
    (<=1152 class-0 + <=1152 class-1 sources per block). Node (core r,
    partition p, block b) has new id r*6272 + p*49 + b, so per-core DRAM
    slices load/store contiguously.
  - y lives in DRAM as [N, 128] bf16 rows (features in cols 0:64, cols 64:128
    never transferred), so a dma_gather with elem_size=64/elem_step=128 moves
    exactly 128B per edge and int16 indices cover each 25088-row half.
  - Per round: AllGather y (1.57MB/core), then one dma_gather per (7-block
    group, source class) fetches 63 tiles x 128 edges of source features.
  - Per edge tile: DVE builds the scaled one-hot onehot[e,j] = (rl==j)*w,
    TensorE accumulates psum[j,c] += onehot^T @ y_g over the 18 tiles.
  - deg from a node-major padded weight layout (one free-axis reduce);
    dinv = rsqrt masked at 0. tx_k = dinv*psum feeds theta matmuls,
    y_{k+1} = dinv^2*psum is re-AllGathered.
  - Theta pass: out = x@Tf0 + Tx1@(Tb0+Tb1) + Tx2@(Tf1+Tb2) + Tx3@Tf2 via
    DMA-transposed [64, 6272] lhsT tiles (thetas combined on host).
  - dma_gather is synced purely via explicit dependency edges; the tile
    scheduler then inserts DMA-completion semaphore waits on its DMASW lanes.

The BIR is input-independent, so the NEFF hits the compile cache.
"""

import numpy as np
import ml_dtypes

N_NODES = 50000
N_EDGES = 800000
C = 64
CP = 128                     # padded feature row width (256B stride)
K = 3
P = 128
N_CORES = 8
NB = 49                      # blocks per core
NBLK = NB * N_CORES          # 392 blocks
NHBLK = NBLK // 2            # 196 blocks per class
N_PAD = NBLK * P             # 50176 padded nodes
NHALF = N_PAD // 2           # 25088 nodes per class (int16-indexable)
TBH = 9                      # tiles per source class
TB = 2 * TBH                 # 18 tiles (of 128 edges) per block
CAPP = TBH * P               # 1152 edge capacity per source class per block
S = 48                       # max in-degree supported by the deg layout
SLICE = NB * P               # 6272 nodes per core
T = NB * TB                  # 882 edge tiles per core
GRP = 7                      # blocks per dma_gather group
OHC = 168                    # one-hot tiles cached in SBUF across rounds
NGRP = NB // GRP             # 7 groups
IDXB = TB * P // 16          # 144 int16 idx columns per block

npbf = ml_dtypes.bfloat16

_CACHE = {}


def _balance_class(nodes, dA, dB):
    """Pack `nodes` (exactly NHALF of them) into NHBLK blocks of 128 with
    per-class edge caps. dA/dB are per-node in-edge counts by source class.
    Returns blk[i], slot[i] aligned with `nodes`."""
    tot = dA[nodes] + dB[nodes]
    order = np.argsort(-tot, kind="stable")
    sumA = np.zeros(NHBLK, dtype=np.int64)
    sumB = np.zeros(NHBLK, dtype=np.int64)
    cnt = np.zeros(NHBLK, dtype=np.int64)
    blk = np.empty(len(nodes), dtype=np.int64)
    slot = np.empty(len(nodes), dtype=np.int64)
    INF = 1 << 60
    for i in order:
        node = nodes[i]
        a, b_ = dA[node], dB[node]
        feas = (cnt < P) & (sumA + a <= CAPP) & (sumB + b_ <= CAPP)
        masked = np.where(feas, sumA + sumB, INF)
        j = int(np.argmin(masked))
        if masked[j] == INF:
            masked = np.where(cnt < P, sumA + sumB, INF)
            j = int(np.argmin(masked))
        blk[i] = j
        slot[i] = cnt[j]
        cnt[j] += 1
        sumA[j] += a
        sumB[j] += b_
    assert cnt.max() == P and cnt.min() == P
    assert sumA.max() <= CAPP and sumB.max() <= CAPP, \
        f"class cap overflow A={sumA.max()} B={sumB.max()} > {CAPP}"
    return blk, slot


def _preprocess(x, edge_index, edge_weight, theta_forward, theta_backward):
    row = np.asarray(edge_index[0], dtype=np.int64)
    col = np.asarray(edge_index[1], dtype=np.int64)
    w = np.asarray(edge_weight, dtype=np.float32)
    x = np.asarray(x, dtype=np.float32)

    # class assignment: alternate over in-degree-sorted nodes so both classes
    # get ~equal node counts and edge mass; pads fill to exactly NHALF each.
    deg_tot = np.bincount(row, minlength=N_PAD).astype(np.int64)
    order = np.argsort(-deg_tot[:N_NODES], kind="stable")
    cls = np.empty(N_PAD, dtype=np.int64)
    cls[order[0::2]] = 0
    cls[order[1::2]] = 1
    n0 = int((cls[:N_NODES] == 0).sum())
    pad = np.arange(N_NODES, N_PAD)
    cls[pad[:NHALF - n0]] = 0
    cls[pad[NHALF - n0:]] = 1
    assert (cls == 0).sum() == NHALF and (cls == 1).sum() == NHALF

    # per-destination in-edge counts by source class
    src_cls = cls[col]
    dA = np.zeros(N_PAD, dtype=np.int64)
    dB = np.zeros(N_PAD, dtype=np.int64)
    np.add.at(dA, row[src_cls == 0], 1)
    np.add.at(dB, row[src_cls == 1], 1)

    nodes0 = np.where(cls == 0)[0]
    nodes1 = np.where(cls == 1)[0]
    blk0, slot0 = _balance_class(nodes0, dA, dB)
    blk1, slot1 = _balance_class(nodes1, dA, dB)

    gblk = np.empty(N_PAD, dtype=np.int64)   # global block 0..391
    gslot = np.empty(N_PAD, dtype=np.int64)
    gblk[nodes0] = blk0
    gslot[nodes0] = slot0
    gblk[nodes1] = NHBLK + blk1
    gslot[nodes1] = slot1

    core = gblk // NB
    b_local = gblk % NB
    perm = core * SLICE + gslot * NB + b_local   # perm[old] = new

    new_row = perm[row]
    new_col = perm[col]
    assert (new_col[src_cls == 0] < NHALF).all()
    assert (new_col[src_cls == 1] >= NHALF).all()

    # group edges by (dest block, source class)
    dest_blk = gblk[row]
    key = dest_blk * 2 + src_cls
    edge_order = np.argsort(key, kind="stable")
    key_s = key[edge_order]
    slot_s = gslot[row][edge_order]
    idx_s = new_col[edge_order] - src_cls[edge_order] * NHALF
    w_s = w[edge_order]
    starts = np.searchsorted(key_s, np.arange(2 * NBLK))
    ends = np.searchsorted(key_s, np.arange(2 * NBLK) + 1)

    w_arr = np.zeros((N_CORES, P, T), dtype=np.float32)
    rl_arr = np.zeros((N_CORES, P, T), dtype=np.float32)
    # per core: group-major idx layout: group g holds [A-tiles of its GRP
    # blocks | B-tiles] = 2*GRP*TBH*P idxs, wrapped i -> [i%16, i//16]
    idx_flat = np.zeros((N_CORES, NGRP, 2, GRP, TBH * P), dtype=np.int16)
    wdeg_arr = np.zeros((N_CORES, P, NB * S), dtype=np.float32)

    for bb in range(NBLK):
        r, b = divmod(bb, NB)
        g, bi = divmod(b, GRP)
        for sc in range(2):
            s0, s1 = starts[bb * 2 + sc], ends[bb * 2 + sc]
            n = s1 - s0
            assert n <= CAPP
            idx = np.arange(n)
            t_idx = b * TB + sc * TBH + idx // P
            e_idx = idx % P
            w_arr[r, e_idx, t_idx] = w_s[s0:s1]
            rl_arr[r, e_idx, t_idx] = slot_s[s0:s1]
            idx_flat[r, g, sc, bi, idx] = idx_s[s0:s1]

    idx_flat = idx_flat.reshape(N_CORES, NGRP * 2 * GRP * TBH * P)
    idx_wrap = idx_flat.reshape(N_CORES, -1, 16).transpose(0, 2, 1)
    idx_full = np.tile(np.ascontiguousarray(idx_wrap), (1, 8, 1))

    # node-major deg layout: node new-id g = r*SLICE + p*NB + b
    node_order = np.argsort(new_row, kind="stable")
    nr_s = new_row[node_order]
    wv_s = w[node_order]
    node_starts = np.searchsorted(nr_s, nr_s)
    kk = np.arange(len(nr_s)) - node_starts
    assert kk.max() < S, f"degree overflow: {kk.max() + 1} > {S}"
    core_i = nr_s // SLICE
    s_i = nr_s % SLICE
    slot_i = s_i // NB
    bl_i = s_i % NB
    wdeg_arr[core_i, slot_i, bl_i * S + kk] = wv_s

    # combined thetas
    tf = np.asarray(theta_forward, dtype=np.float32)
    tb = np.asarray(theta_backward, dtype=np.float32)
    th4 = np.stack([tf[0], tb[0] + tb[1], tf[1] + tb[2], tf[2]])  # [4,64,64]
    th4_flat = np.ascontiguousarray(
        th4.transpose(1, 0, 2).reshape(C, 4 * C)).astype(npbf)

    iota = np.tile(np.arange(P, dtype=np.float32)[None, :], (P, 1)).astype(npbf)

    x_pad = np.zeros((N_PAD, C), dtype=np.float32)
    x_pad[perm[:N_NODES]] = x
    in_maps = []
    for r in range(N_CORES):
        in_maps.append({
            "xsl": np.ascontiguousarray(x_pad[r * SLICE:(r + 1) * SLICE]),
            "gidx": np.ascontiguousarray(idx_full[r]),
            "w": np.ascontiguousarray(w_arr[r]),
            "rl": np.ascontiguousarray(rl_arr[r]),
            "wdeg": np.ascontiguousarray(wdeg_arr[r]),
            "iota": iota,
            "th4": th4_flat,
        })
    return in_maps, perm



def _dma_gather_small(eng, bass, mybir, out_ap, in_ap, idxs_ap, num_idxs,
                      elem_size, elem_step, single_packet=False):
    """bass.dma_gather minus the elem%256B assert (non-transpose DRAM src).

    in_ap rows are elem_step elements apart; only elem_size are moved (the
    256B-multiple restriction only applies to transpose mode; 128B payloads
    on a 256B stride are HW-validated)."""
    from concourse import ap_utils
    from concourse.bass import exact_div
    assert idxs_ap.dtype == mybir.dt.int16
    assert in_ap.dtype == out_ap.dtype
    assert in_ap.space == bass.MemorySpace.DRAM
    assert idxs_ap.space == bass.MemorySpace.SBUF
    assert out_ap.space == bass.MemorySpace.SBUF
    assert ap_utils.ap_is_contiguous(out_ap.ap[1:])
    assert ap_utils.ap_is_contiguous(idxs_ap.ap[1:])
    assert in_ap.ap[-1][1] == out_ap.ap[-1][1] == elem_size
    assert out_ap.ap[0][1] * out_ap.ap[1][1] == num_idxs
    assert in_ap.ap[0][0] == elem_step
    stride_bytes = elem_step * mybir.dt.size(in_ap.dtype)
    stride_bytes_256 = exact_div(stride_bytes, 256)
    _in_ap = eng.lower_ap_dma(in_ap, for_custom_bir_dma=True)
    _idxs_ap = eng.lower_ap(idxs_ap)
    _out_ap = eng.lower_ap(out_ap)
    return eng.add_instruction(
        mybir.InstDMAGatherAnt(
            name=eng.bass.get_next_instruction_name(),
            ins=[*_in_ap, _idxs_ap,
                 eng.lower_val_access(eng.to_reg(num_idxs))],
            outs=[_out_ap],
            transpose=False,
            num_idxs=num_idxs,
            elem_size=elem_size,
            stride_bytes_256=stride_bytes_256,
            gen_mode=0,
            single_packet=single_packet,
            queue_num=0,
            sbuf_tokens_per_rank=0,
            sbuf_free_dim_per_rank=0,
            sbuf_free_dim_pad_per_rank=0,
            sbuf_byte_offset=0,
        ))


def build_nc():
    """Build and compile the Bacc program (input-data independent)."""
    import concourse.bacc as bacc
    import concourse.bass as bass
    import concourse.mybir as mybir
    import concourse.tile as tile
    from bass_rust import add_dep_helper

    DT = mybir.dt.bfloat16
    F32 = mybir.dt.float32

    nc = bacc.Bacc("TRN2", target_bir_lowering=False, debug=False,
                   num_devices=N_CORES)
    xsl_d = nc.dram_tensor("xsl", [SLICE, C], F32, kind="ExternalInput")
    gidx_d = nc.dram_tensor("gidx", [P, NB * IDXB], mybir.dt.int16,
                            kind="ExternalInput")
    w_d = nc.dram_tensor("w", [P, T], F32, kind="ExternalInput")
    rl_d = nc.dram_tensor("rl", [P, T], F32, kind="ExternalInput")
    wdeg_d = nc.dram_tensor("wdeg", [P, NB * S], F32, kind="ExternalInput")
    iota_d = nc.dram_tensor("iota", [P, P], DT, kind="ExternalInput")
    th4_d = nc.dram_tensor("th4", [C, 4 * C], DT, kind="ExternalInput")
    out_d = nc.dram_tensor("out", [SLICE, C], F32, kind="ExternalOutput")

    ag_in = [nc.dram_tensor(f"ag_in{k}", [SLICE, CP], DT, kind="Internal")
             for k in range(K)]
    ag_out = [nc.dram_tensor(f"ag_out{k}", [N_PAD, CP], DT, kind="Internal",
                             addr_space="Shared") for k in range(K)]
    # block-major ([b*128+p, c]) theta sources
    xbf_d = nc.dram_tensor("xbf", [SLICE, C], DT, kind="Internal")
    tx_d = [nc.dram_tensor(f"tx{k+1}", [SLICE, C], DT, kind="Internal")
            for k in range(K)]

    with tile.TileContext(nc) as tc:
        with tc.tile_pool(name="const", bufs=1) as cp, \
             tc.tile_pool(name="oh", bufs=8) as ohp, \
             tc.tile_pool(name="psum", bufs=6, space="PSUM") as pp, \
             tc.tile_pool(name="psum2", bufs=2, space="PSUM") as pp2:
            gidx_sb = cp.tile([P, NB * IDXB], mybir.dt.int16, name="gidx_sb")
            w_sb = cp.tile([P, T], F32, name="w_sb")
            rl_sb = cp.tile([P, T], F32, name="rl_sb")
            wdeg_sb = cp.tile([P, NB * S], F32, name="wdeg_sb")
            iota_sb = cp.tile([P, P], DT, name="iota_sb")
            th4_sb = cp.tile([C, 4 * C], DT, name="th4_sb")
            x_all = cp.tile([P, NB, C], F32, name="x_all")
            out_all = x_all                   # head-only / tail-only overlap
            xbf_all = cp.tile([P, NB * C], DT, name="xbf_all")
            y_all = cp.tile([P, NB, C], DT, name="y_all")
            tx_all = cp.tile([P, NB, C], DT, name="tx_all")
            srcT = [cp.tile([C, SLICE], DT, tag=f"srcT{j}",
                            name=f"srcT{j}") for j in range(4)]
            ybuf = [cp.tile([P, GRP * TB, C], DT, tag=f"ybuf{i}",
                            name=f"ybuf{i}") for i in range(3)]
            ohc_sb = cp.tile([P, OHC, P], DT, name="ohc_sb")

            nc.sync.dma_start(gidx_sb[:], gidx_d[:])
            nc.sync.dma_start(w_sb[:], w_d[:])
            nc.sync.dma_start(rl_sb[:], rl_d[:])
            nc.sync.dma_start(wdeg_sb[:], wdeg_d[:])
            nc.sync.dma_start(iota_sb[:], iota_d[:])
            nc.sync.dma_start(th4_sb[:], th4_d[:])
            # x: slice row p*NB+b -> partition p, free (b, c); contiguous rows
            nc.sync.dma_start(
                x_all[:], xsl_d[:].rearrange("(p b) c -> p b c", p=P))

            # ---- degree + dinv (all local) ----
            deg_sb = cp.tile([P, NB], F32, name="deg_sb")
            dinv_sb = cp.tile([P, NB], F32, name="dinv_sb")
            dinv2_sb = cp.tile([P, NB], F32, name="dinv2_sb")
            mask_sb = cp.tile([P, NB], F32, name="mask_sb")
            sqrt_sb = cp.tile([P, NB], F32, name="sqrt_sb")
            nc.vector.tensor_reduce(
                out=deg_sb[:],
                in_=wdeg_sb[:].rearrange("p (b s) -> p b s", s=S),
                axis=mybir.AxisListType.X, op=mybir.AluOpType.add)
            nc.scalar.activation(out=sqrt_sb[:], in_=deg_sb[:],
                                 func=mybir.ActivationFunctionType.Sqrt)
            nc.vector.reciprocal(out=dinv_sb[:], in_=sqrt_sb[:])
            nc.vector.tensor_scalar(out=mask_sb[:], in0=deg_sb[:],
                                    scalar1=0.0, scalar2=None,
                                    op0=mybir.AluOpType.is_gt)
            nc.vector.tensor_tensor(out=dinv_sb[:], in0=dinv_sb[:],
                                    in1=mask_sb[:], op=mybir.AluOpType.mult)
            nc.vector.tensor_tensor(out=dinv2_sb[:], in0=dinv_sb[:],
                                    in1=dinv_sb[:], op=mybir.AluOpType.mult)

            # ---- y0 = dinv * x; xbf = bf16(x) ----
            nc.vector.tensor_copy(
                out=xbf_all[:], in_=x_all[:].rearrange("p b c -> p (b c)"))
            for b in range(NB):
                nc.vector.tensor_scalar(out=y_all[:, b, :], in0=x_all[:, b, :],
                                        scalar1=dinv_sb[:, b:b + 1],
                                        scalar2=None,
                                        op0=mybir.AluOpType.mult)
            nc.sync.dma_start(
                ag_in[0][:].rearrange("(p b) c -> p b c", p=P)[:, :, 0:C],
                y_all[:])
            # block-major write: row b*128+p <- xbf_all[p, b*C:(b+1)*C]
            nc.sync.dma_start(
                xbf_d[:].rearrange("(b p) c -> p b c", p=P),
                xbf_all[:].rearrange("p (b c) -> p b c", c=C))
            nc.sync.dma_start_transpose(srcT[0][:], xbf_d[:])

            # ---- propagation rounds ----
            prev_gather = None
            prev_round_lasts = None
            for k in range(K):
                nc.gpsimd.collective_compute(
                    "AllGather", mybir.AluOpType.bypass,
                    replica_groups=[list(range(N_CORES))],
                    ins=[ag_in[k][:]], outs=[ag_out[k][:]])

                block_last_mms = {}      # g -> [last mm of each block]
                for g in range(NGRP):
                    blocks = list(range(g * GRP, (g + 1) * GRP))
                    nidx = GRP * TBH * P              # per class
                    yb = ybuf[g % 3]
                    gis = []
                    for sc in range(2):
                        gi = _dma_gather_small(
                            nc.gpsimd, bass, mybir,
                            out_ap=yb[:, sc * GRP * TBH:
                                      (sc + 1) * GRP * TBH, :],
                            in_ap=ag_out[k][sc * NHALF:(sc + 1) * NHALF, 0:C],
                            idxs_ap=gidx_sb[
                                :, (g * 2 + sc) * (nidx // 16):
                                (g * 2 + sc + 1) * (nidx // 16)],
                            num_idxs=nidx,
                            elem_size=C,
                            elem_step=CP,
                        )
                        if prev_gather is not None:
                            add_dep_helper(gi.ins, prev_gather.ins,
                                           reason="gather stream order")
                        if g >= 3:
                            for pm in block_last_mms[g - 3]:
                                add_dep_helper(gi.ins, pm.ins,
                                               reason="ybuf WAR")
                        elif prev_round_lasts is not None:
                            # last round-(k-1) group using buffer g%3
                            gp = g + 3 * ((NGRP - 1 - g) // 3)
                            for pm in prev_round_lasts[gp]:
                                add_dep_helper(gi.ins, pm.ins,
                                               reason="ybuf WAR xround")
                        prev_gather = gi
                        gis.append(gi)

                    lasts = []
                    for bi, b in enumerate(blocks):
                        psum = pp.tile([P, C], F32, tag="ps", name="ps")
                        for t in range(TB):
                            gg = b * TB + t
                            if gg < OHC:
                                oh = ohc_sb[:, gg, :]
                                if k == 0:
                                    nc.vector.tensor_scalar(
                                        out=oh, in0=iota_sb[:],
                                        scalar1=rl_sb[:, gg:gg + 1],
                                        scalar2=w_sb[:, gg:gg + 1],
                                        op0=mybir.AluOpType.is_equal,
                                        op1=mybir.AluOpType.mult)
                            else:
                                oh = ohp.tile([P, P], DT, tag="oh",
                                              name="oh")[:]
                                nc.vector.tensor_scalar(
                                    out=oh, in0=iota_sb[:],
                                    scalar1=rl_sb[:, gg:gg + 1],
                                    scalar2=w_sb[:, gg:gg + 1],
                                    op0=mybir.AluOpType.is_equal,
                                    op1=mybir.AluOpType.mult)
                            sc = 0 if t < TBH else 1
                            tt = sc * GRP * TBH + bi * TBH + (t - sc * TBH)
                            mm = nc.tensor.matmul(
                                psum[:], lhsT=oh,
                                rhs=yb[:, tt, :],
                                start=(t == 0), stop=(t == TB - 1))
                            add_dep_helper(mm.ins, gis[sc].ins,
                                           reason="ybuf RAW")
                        lasts.append(mm)
                        nc.scalar.activation(
                            out=tx_all[:, b, :], in_=psum[:],
                            func=mybir.ActivationFunctionType.Copy,
                            scale=dinv_sb[:, b:b + 1])
                        if k < K - 1:
                            nc.scalar.activation(
                                out=y_all[:, b, :], in_=psum[:],
                                func=mybir.ActivationFunctionType.Copy,
                                scale=dinv2_sb[:, b:b + 1])
                    block_last_mms[g] = lasts
                prev_round_lasts = block_last_mms
                # round tail: flush tx (block-major) + transpose; y -> ag_in
                nc.sync.dma_start(
                    tx_d[k][:].rearrange("(b p) c -> p b c", p=P),
                    tx_all[:])
                nc.sync.dma_start_transpose(srcT[k + 1][:], tx_d[k][:])
                if k < K - 1:
                    nc.sync.dma_start(
                        ag_in[k + 1][:].rearrange(
                            "(p b) c -> p b c", p=P)[:, :, 0:C],
                        y_all[:])

            # ---- theta pass ----
            for b in range(NB):
                psum_o = pp2.tile([P, C], F32, tag="pso", name="pso")
                for j in range(4):
                    nc.tensor.matmul(psum_o[:],
                                     lhsT=srcT[j][:, b * P:(b + 1) * P],
                                     rhs=th4_sb[:, j * C:(j + 1) * C],
                                     start=(j == 0), stop=(j == 3))
                nc.vector.tensor_copy(out=out_all[:, b, :], in_=psum_o[:])
            nc.sync.dma_start(
                out_d[:].rearrange("(p b) c -> p b c", p=P), out_all[:])

    nc.compile()
    return nc


def _get_nc():
    if "nc" not in _CACHE:
        _CACHE["nc"] = build_nc()
    return _CACHE["nc"]


def kernel(x, edge_index, edge_weight, theta_forward, theta_backward):
    from concourse.bass_utils import run_bass_kernel_spmd

    in_maps, perm = _preprocess(x, edge_index, edge_weight,
                                theta_forward, theta_backward)
    nc = _get_nc()
    res = run_bass_kernel_spmd(nc, in_maps, core_ids=list(range(N_CORES)))
    out_pad = np.concatenate([res.results[r]["out"] for r in range(N_CORES)],
                             axis=0)
    return np.ascontiguousarray(out_pad[perm[:N_NODES]]).astype(np.float32)
